# revision 1
# baseline (speedup 1.0000x reference)
"""CVTGAD loss kernel for 8 TRN2 NeuronCores.

Math (matches the jax reference):
  l_node[b] = mean_i [ lse_j(hf_i.hsN_j * 2*invf_i) - s_ii ]   per graph (128x128 InfoNCE)
  l_graph   = InfoNCE over pooled graph embeddings (512x512)
  out = (std(l_node)+1e-6) * mean(l_node) + (std(l_graph)+1e-6) * mean(l_graph)

Sharding: 64 graphs (8192 node rows) per core; h_s_final replicated (rolled per
core so each core's own graphs sit at columns 0:64, making the SPMD diag mask
core-independent). Device computes per-graph node-loss columns [128,64] and
l_graph [64]; host does the tiny std/mean/weighted-sum epilogue.

Kernel strategy per core:
  - SWDGE DMA-cast loads f32->bf16 (HBM reads are the 47us roofline).
  - Row sumsq: ACT Square+accum_out (hf), DVE tensor_tensor_reduce (hs).
  - inv norms as exp(-0.5*ln(x)) -- single ACT table set (natural_log_exp).
  - hs scaled by 1/|hs| (DVE tensor_scalar, bf16); hf's 2/(tau=0.5 twist)/|hf|
    folded into the per-partition scale AP of the ACT Exp.
  - Transposes via batched x-bar DMA transpose (one 512KB DmaTranspose per
    tensor per 8-graph block -> [d, graph, node] chunks, no PSUM round-trip).
  - Per-graph bf16 Gram matmuls (K=256 in 2 chunks) -> PSUM f32.
  - ACT Exp reads PSUM with per-graph scale AP, accum_out = rowsum; diagonal
    exp via DVE mask-mul + tensor_scalar accum; l = ln(rowsum * recip(diag)).
  - All activations pinned to the natural_log_exp_and_others table set
    (single ACT_TABLE_LOAD; the default picker thrashes ~2.7us reloads).
"""

import numpy as np

B = 512
NPER = 128
D = 256
NCORES = 8
GPC = B // NCORES      # 64 graphs per core
BLK = 8                # graphs per DMA block
NBLK = GPC // BLK
QG = 4                 # graphs per PSUM group (transpose/gram granularity)
TAU = 0.5
LN_INV_TAU = float(np.log(1.0 / TAU))

_CACHE = {}


def _build():
    import os
    import ml_dtypes
    import concourse.bacc as bacc
    import concourse.tile as tile
    import concourse.mybir as mybir
    import concourse.hw_specs as hw_specs
    from concourse._compat import get_trn_type

    # Pin every activation to the one table set that has Exp+Ln+Square+Copy,
    # so the compiler emits a single ACT_TABLE_LOAD instead of thrashing
    # (each reload costs ~2.7us and the default picker alternates sets).
    if not getattr(hw_specs, "_nle_patched", False):
        _orig_tables = hw_specs.get_activation_tables

        def _only_nle(arch):
            t = _orig_tables(arch)
            keep = "natural_log_exp_and_others"
            return {k: (v if k == keep else set()) for k, v in t.items()}

        hw_specs.get_activation_tables = _only_nle
        bacc.get_activation_tables = _only_nle
        hw_specs._nle_patched = True

    f32 = mybir.dt.float32
    bf16 = mybir.dt.bfloat16
    AF = mybir.ActivationFunctionType
    ALU = mybir.AluOpType

    nc = bacc.Bacc(get_trn_type() or "TRN2", target_bir_lowering=False, debug=True)

    hf = nc.declare_dram_parameter("hf", [GPC * NPER, D], f32, isOutput=False)
    hs = nc.declare_dram_parameter("hs", [GPC * NPER, D], f32, isOutput=False)
    hff = nc.declare_dram_parameter("hff", [GPC, D], f32, isOutput=False)
    hsf = nc.declare_dram_parameter("hsf", [B, D], f32, isOutput=False)
    out_node = nc.declare_dram_parameter("out_node", [NPER, GPC], f32, isOutput=True)
    out_graph = nc.declare_dram_parameter("out_graph", [GPC, 1], f32, isOutput=True)

    eye_dram = nc.inline_tensor(np.eye(128, dtype=ml_dtypes.bfloat16), "eye_bf")

    with tile.TileContext(nc) as tc:
        with (
            tc.tile_pool(name="consts", bufs=1) as consts,
            tc.tile_pool(name="cols", bufs=1) as colsp,
            tc.tile_pool(name="loads", bufs=int(os.environ.get("K_LOADS", "4"))) as loads,
            tc.tile_pool(name="work", bufs=int(os.environ.get("K_WORK", "3"))) as work,
            tc.tile_pool(name="scr", bufs=int(os.environ.get("K_SCR", "2"))) as scr,
        ):
            ident = consts.tile([128, 128], bf16)
            nc.sync.dma_start(out=ident, in_=eye_dram[:, :])
            lntau_c = consts.tile([128, 1], f32)
            nc.vector.memset(lntau_c, LN_INV_TAU)

            # per-graph column stats [128, GPC] f32
            ssq_f = colsp.tile([128, GPC], f32)
            ssq_s = colsp.tile([128, GPC], f32)
            invs_c = colsp.tile([128, GPC], f32)
            invf2_c = colsp.tile([128, GPC], f32)
            rowsum_c = colsp.tile([128, GPC], f32)
            dexp_c = colsp.tile([128, GPC], f32)
            ln_scr = colsp.tile([128, GPC], f32)
            l_cols = colsp.tile([128, GPC], f32)

            # ---------------- graph-level loss (own PSUM scope) ----------------
            with (
                tc.tile_pool(name="fin", bufs=1) as fin,
                tc.tile_pool(name="fpsum", bufs=1, space="PSUM") as fpsum,
            ):
                hff_bf = fin.tile([GPC, D], bf16)
                nc.gpsimd.dma_start(out=hff_bf, in_=hff[:, :])
                hsf_bf = fin.tile([128, 4, D], bf16)
                nc.gpsimd.dma_start(
                    out=hsf_bf, in_=hsf[:, :].rearrange("(r p) d -> p r d", p=128)
                )

                ssq_ff = fin.tile([GPC, 1], f32)
                sqf_scr = fin.tile([GPC, D], bf16)
                nc.scalar.activation(sqf_scr, hff_bf, AF.Square, accum_out=ssq_ff)

                ssq_sf = fin.tile([128, 4], f32)
                sqs_scr = fin.tile([128, D], bf16)
                for r in range(4):
                    nc.scalar.activation(
                        sqs_scr, hsf_bf[:, r, :], AF.Square,
                        accum_out=ssq_sf[:, r : r + 1],
                    )

                lnf_scr = fin.tile([128, 4], f32)
                invs_f = fin.tile([128, 4], f32)
                nc.scalar.activation(lnf_scr, ssq_sf, AF.Ln)
                nc.scalar.activation(invs_f, lnf_scr, AF.Exp, scale=-0.5)

                lnf2_scr = fin.tile([GPC, 1], f32)
                invf2_f = fin.tile([GPC, 1], f32)
                nc.scalar.activation(lnf2_scr, ssq_ff, AF.Ln)
                nc.scalar.activation(
                    invf2_f, lnf2_scr, AF.Exp, scale=-0.5, bias=lntau_c[:GPC]
                )

                hsN_f = fin.tile([128, 4, D], bf16)
                for r in range(4):
                    nc.vector.tensor_scalar_mul(
                        hsN_f[:, r, :], hsf_bf[:, r, :], invs_f[:, r : r + 1]
                    )

                hffT = fin.tile([128, 2, GPC], bf16)
                nc.sync.dma_start(out=hffT, in_=hff_bf, transpose=True)
                hsfT = fin.tile([128, 4, 2, 128], bf16)
                nc.sync.dma_start(out=hsfT, in_=hsN_f, transpose=True)

                sfin_ps = fpsum.tile([GPC, 512], f32)
                for c in range(2):
                    nc.tensor.matmul(
                        sfin_ps, hffT[:, c, :], hsfT[:, :, c, :],
                        start=(c == 0), stop=(c == 1),
                    )

                expf = fin.tile([GPC, 512], bf16)
                rowsum_f = fin.tile([GPC, 1], f32)
                nc.scalar.activation(
                    expf, sfin_ps, AF.Exp, scale=invf2_f, accum_out=rowsum_f
                )

                maskf = fin.tile([GPC, 512], bf16)
                nc.vector.memset(maskf, 0.0)
                nc.vector.tensor_copy(maskf[:, 0:GPC], ident[:GPC, :GPC])
                maskd_f = fin.tile([GPC, 512], bf16)
                nc.vector.tensor_tensor(maskd_f, expf, maskf, op=ALU.mult)
                dexp_f = fin.tile([GPC, 1], f32)
                tsf_scr = fin.tile([GPC, 512], bf16)
                nc.vector.tensor_scalar(
                    tsf_scr, maskd_f, 1.0, 0.0,
                    op0=ALU.mult, op1=ALU.add, accum_out=dexp_f,
                )

                recip_f = fin.tile([GPC, 1], f32)
                nc.vector.reciprocal(recip_f, dexp_f)
                ratio_f = fin.tile([GPC, 1], f32)
                nc.vector.tensor_tensor(ratio_f, rowsum_f, recip_f, op=ALU.mult)
                lg = fin.tile([GPC, 1], f32)
                nc.scalar.activation(lg, ratio_f, AF.Ln)
                nc.sync.dma_start(out=out_graph[:, :], in_=lg)

            # ---------------- node-level loss ----------------
            hf_r = hf[:, :].rearrange("(g p) d -> p g d", p=128)
            hs_r = hs[:, :].rearrange("(g p) d -> p g d", p=128)
            with (
                tc.tile_pool(name="spsum", bufs=int(os.environ.get("K_SPSUM", "2")), space="PSUM") as spsum,
            ):
                for b in range(NBLK):
                    bs = slice(b * BLK, (b + 1) * BLK)
                    hf_bf = loads.tile([128, BLK, D], bf16, tag="hf_bf")
                    nc.gpsimd.dma_start(out=hf_bf, in_=hf_r[:, bs, :])
                    hs_bf = loads.tile([128, BLK, D], bf16, tag="hs_bf")
                    nc.gpsimd.dma_start(out=hs_bf, in_=hs_r[:, bs, :])

                    for g in range(BLK):
                        gg = b * BLK + g
                        # hf sumsq: ACT for most graphs, DVE for some,
                        # to balance engine busy time.
                        if g < 5:
                            sq_scr = scr.tile([128, D], bf16, tag="sqf")
                            nc.scalar.activation(
                                sq_scr, hf_bf[:, g, :], AF.Square,
                                accum_out=ssq_f[:, gg : gg + 1],
                            )
                        else:
                            sqf_t = scr.tile([128, D], bf16, tag="sqfv")
                            nc.vector.tensor_tensor(
                                sqf_t, hf_bf[:, g, :], hf_bf[:, g, :],
                                op=ALU.mult,
                            )
                            tsf_scr = scr.tile([128, D], bf16, tag="tsfv")
                            nc.vector.tensor_scalar(
                                tsf_scr, sqf_t, 1.0, 0.0, op0=ALU.mult,
                                op1=ALU.add, accum_out=ssq_f[:, gg : gg + 1],
                            )
                        sqs_t = scr.tile([128, D], bf16, tag="sqs")
                        nc.vector.tensor_tensor(
                            sqs_t, hs_bf[:, g, :], hs_bf[:, g, :], op=ALU.mult
                        )
                        ts_scr = scr.tile([128, D], bf16, tag="tss")
                        nc.vector.tensor_scalar(
                            ts_scr, sqs_t, 1.0, 0.0, op0=ALU.mult, op1=ALU.add,
                            accum_out=ssq_s[:, gg : gg + 1],
                        )

                    nc.scalar.activation(ln_scr[:, bs], ssq_s[:, bs], AF.Ln)
                    nc.scalar.activation(invs_c[:, bs], ln_scr[:, bs], AF.Exp, scale=-0.5)
                    nc.scalar.activation(ln_scr[:, bs], ssq_f[:, bs], AF.Ln)
                    nc.scalar.activation(
                        invf2_c[:, bs], ln_scr[:, bs], AF.Exp,
                        scale=-0.5, bias=lntau_c,
                    )

                    hsN = work.tile([128, BLK, D], bf16, tag="hsN")
                    for g in range(BLK):
                        gg = b * BLK + g
                        nc.vector.tensor_scalar_mul(
                            hsN[:, g, :], hs_bf[:, g, :], invs_c[:, gg : gg + 1]
                        )

                    tT_f = work.tile([128, BLK, 2, 128], bf16, tag="tT_f")
                    tT_s = work.tile([128, BLK, 2, 128], bf16, tag="tT_s")
                    nc.sync.dma_start(out=tT_f, in_=hf_bf, transpose=True)
                    nc.sync.dma_start(out=tT_s, in_=hsN, transpose=True)
                    for q in range(BLK // QG):
                        s_ps = spsum.tile([128, QG, 128], f32, tag="s_ps")
                        for j in range(QG):
                            g = q * QG + j
                            gg = b * BLK + g
                            for c in range(2):
                                nc.tensor.matmul(
                                    s_ps[:, j, :],
                                    tT_f[:, g, c, :],
                                    tT_s[:, g, c, :],
                                    start=(c == 0), stop=(c == 1),
                                )
                            exp_scr = scr.tile([128, 128], bf16, tag="exps")
                            nc.scalar.activation(
                                exp_scr, s_ps[:, j, :], AF.Exp,
                                scale=invf2_c[:, gg : gg + 1],
                                accum_out=rowsum_c[:, gg : gg + 1],
                            )
                            diag_scr = scr.tile([128, 128], bf16, tag="diags")
                            nc.vector.tensor_tensor(
                                diag_scr, exp_scr, ident, op=ALU.mult
                            )
                            dts_scr = scr.tile([128, 128], bf16, tag="dtss")
                            nc.vector.tensor_scalar(
                                dts_scr, diag_scr, 1.0, 0.0,
                                op0=ALU.mult, op1=ALU.add,
                                accum_out=dexp_c[:, gg : gg + 1],
                            )



                recip_c = colsp.tile([128, GPC], f32)
                nc.vector.reciprocal(recip_c, dexp_c)
                ratio_c = colsp.tile([128, GPC], f32)
                nc.vector.tensor_tensor(ratio_c, rowsum_c, recip_c, op=ALU.mult)
                nc.scalar.activation(l_cols, ratio_c, AF.Ln)
                nc.sync.dma_start(out=out_node[:, :], in_=l_cols)

    nc.compile()
    return nc


def _get_nc():
    if "nc" not in _CACHE:
        _CACHE["nc"] = _build()
    return _CACHE["nc"]


def _run(in_maps, **kwargs):
    from concourse.bass_utils import run_bass_kernel_spmd

    return run_bass_kernel_spmd(_get_nc(), in_maps, core_ids=list(range(NCORES)), **kwargs)


def make_in_maps(h_f_final, h_s_final, h_f, h_s):
    h_f = np.ascontiguousarray(np.asarray(h_f, dtype=np.float32))
    h_s = np.ascontiguousarray(np.asarray(h_s, dtype=np.float32))
    h_f_final = np.ascontiguousarray(np.asarray(h_f_final, dtype=np.float32))
    h_s_final = np.ascontiguousarray(np.asarray(h_s_final, dtype=np.float32))
    rows = GPC * NPER
    in_maps = []
    for c in range(NCORES):
        in_maps.append(
            {
                "hf": h_f[c * rows : (c + 1) * rows],
                "hs": h_s[c * rows : (c + 1) * rows],
                "hff": h_f_final[c * GPC : (c + 1) * GPC],
                "hsf": np.ascontiguousarray(np.roll(h_s_final, -GPC * c, axis=0)),
            }
        )
    return in_maps


def finish(results):
    l_node = np.concatenate(
        [r["out_node"].astype(np.float64).mean(axis=0) for r in results]
    )
    l_graph = np.concatenate([r["out_graph"][:, 0].astype(np.float64) for r in results])
    lam1 = l_node.std() + 1e-6
    lam2 = l_graph.std() + 1e-6
    return np.float32(lam1 * l_node.mean() + lam2 * l_graph.mean())


def kernel(h_f_final, h_s_final, h_f, h_s, batch=None, **_unused):
    res = _run(make_in_maps(h_f_final, h_s_final, h_f, h_s))
    return finish(res.results)



# revision 6
# speedup vs baseline: 1.5988x; 1.5988x over previous
"""CVTGAD loss kernel for 8 TRN2 NeuronCores (v3: host-transposed sharding).

Math (matches the jax reference):
  l_node[b] = mean_i [ lse_j(s_ij) - s_ii ],  s_ij = hf_i.hs_j * 2*invf_i*invs_j
  l_graph   = InfoNCE over pooled graph embeddings
  out = (std(l_node)+1e-6) * mean(l_node) + (std(l_graph)+1e-6) * mean(l_graph)

Sharding choice: 64 contiguous graphs per core; the per-core node tensors are
laid out feature-major ([D, nodes] -- a host-side sharding/layout transform),
so the contraction dim d sits on SBUF partitions directly:
  - no on-device transposes (the old DMA x-bar transposes were ~30us of
    serialized DMA-engine time per core),
  - per-node sumsq comes nearly free on the idle TensorE (squared tile as
    stationary x ones column -> ssq columns in PSUM),
  - hs-norm scaling uses a TensorE-replicated row tile: colrep = ones*invs
    (per-partition TSP) -> PE transpose -> bf16 PSUM tile with invs[j]
    replicated across partitions, consumed in-place by the DVE scale TT at
    2x (psum bf16 keeps the 2-byte fast path).
  - hf-norm (x 1/tau) folds into the per-graph ACT exp scale AP, as before.

Per-core pipeline (4 chunks x 16 graphs): SWDGE cast loads f32->bf16 ->
squares (hf: DVE TT, hs: ACT Square) -> PE ssq column matmuls -> ACT ln/exp
inv-norms -> colrep TSP + PE repl transpose -> DVE scale TT (hs) -> per-graph
Gram (K=256, 2 matmuls) -> per-graph ACT exp (scale AP) -> DVE rowsum TSP
accums -> diag via Pool ident-mask TT + DVE TSP accums.
Host does the tiny std/mean/weighted-sum epilogue (as before).
"""

import os
import numpy as np

B = 512
NPER = 128
D = 256
NCORES = 8
GPC = B // NCORES      # 64 graphs per core
CHG = 16               # graphs per load chunk
NCH = GPC // CHG       # 4 chunks
QG = 4                 # graphs per PSUM/exp group
TAU = 0.5
LN_INV_TAU = float(np.log(1.0 / TAU))

_CACHE = {}


def _build():
    import ml_dtypes
    import concourse.bacc as bacc
    import concourse.tile as tile
    import concourse.mybir as mybir
    import concourse.hw_specs as hw_specs
    from concourse._compat import get_trn_type

    # Pin every activation to the one table set that has Exp+Ln+Square+Copy,
    # so the compiler emits a single ACT_TABLE_LOAD instead of thrashing.
    if not getattr(hw_specs, "_nle_patched", False):
        _orig_tables = hw_specs.get_activation_tables

        def _only_nle(arch):
            t = _orig_tables(arch)
            keep = "natural_log_exp_and_others"
            return {k: (v if k == keep else set()) for k, v in t.items()}

        hw_specs.get_activation_tables = _only_nle
        bacc.get_activation_tables = _only_nle
        hw_specs._nle_patched = True

    f32 = mybir.dt.float32
    bf16 = mybir.dt.bfloat16
    AF = mybir.ActivationFunctionType
    ALU = mybir.AluOpType

    nc = bacc.Bacc(get_trn_type() or "TRN2", target_bir_lowering=False, debug=True)

    # Host-transposed feature-major layouts.
    hfT = nc.declare_dram_parameter("hfT", [D, GPC * NPER], f32, isOutput=False)
    hsT = nc.declare_dram_parameter("hsT", [D, GPC * NPER], f32, isOutput=False)
    hffT = nc.declare_dram_parameter("hffT", [D, GPC], f32, isOutput=False)
    hsfT = nc.declare_dram_parameter("hsfT", [D, B], f32, isOutput=False)
    out_node = nc.declare_dram_parameter("out_node", [NPER, GPC], f32, isOutput=True)
    out_graph = nc.declare_dram_parameter("out_graph", [GPC, 1], f32, isOutput=True)

    eye_dram = nc.inline_tensor(np.eye(128, dtype=ml_dtypes.bfloat16), "eye_bf")

    SQS_ON_ACT = os.environ.get("K_SQS", "act") == "act"
    SQF_ON_ACT = os.environ.get("K_SQF", "dve") == "act"
    DIAG_POOL = os.environ.get("K_DIAG", "pool") == "pool"

    with tile.TileContext(nc) as tc:
        with (
            tc.tile_pool(name="consts", bufs=1) as consts,
            tc.tile_pool(name="cols", bufs=1) as colsp,
            tc.tile_pool(name="loads", bufs=int(os.environ.get("K_LOADS", "2"))) as loads,
            tc.tile_pool(name="work", bufs=int(os.environ.get("K_WORK", "2"))) as work,
            tc.tile_pool(name="scr", bufs=int(os.environ.get("K_SCR", "3"))) as scr,
        ):
            ident = consts.tile([128, 128], bf16)
            nc.sync.dma_start(out=ident, in_=eye_dram[:, :])
            identx = consts.tile([128, QG, 128], bf16)
            for j in range(QG):
                nc.vector.tensor_copy(identx[:, j, :], ident)
            ones_col = consts.tile([128, 1], bf16)
            nc.vector.memset(ones_col, 1.0)
            ones_sq = consts.tile([128, 128], bf16)
            nc.vector.memset(ones_sq, 1.0)
            lntau_c = consts.tile([128, 1], f32)
            nc.vector.memset(lntau_c, LN_INV_TAU)

            # per-graph column stats [128, GPC]
            rowsum_c = colsp.tile([128, GPC], f32)
            dexp_c = colsp.tile([128, GPC], f32)
            ssqf_s = colsp.tile([128, GPC], f32)
            ssqs_s = colsp.tile([128, GPC], f32)
            invf2_col = colsp.tile([128, GPC], f32)
            invs_col = colsp.tile([128, GPC], f32)
            ln_scr = colsp.tile([128, GPC], f32)

            # ---------------- graph-level loss (own PSUM scope) ----------------
            with (
                tc.tile_pool(name="fin", bufs=1) as fin,
                tc.tile_pool(name="fpsum", bufs=1, space="PSUM") as fpsum,
            ):
                hffT_bf = fin.tile([128, 2, GPC], bf16)
                nc.gpsimd.dma_start(
                    out=hffT_bf, in_=hffT[:, :].rearrange("(c p) n -> p c n", p=128)
                )
                hsfT_bf = fin.tile([128, 2, B], bf16)
                nc.gpsimd.dma_start(
                    out=hsfT_bf, in_=hsfT[:, :].rearrange("(c p) n -> p c n", p=128)
                )

                sqff = fin.tile([128, 2, GPC], bf16)
                nc.vector.tensor_tensor(sqff, hffT_bf, hffT_bf, op=ALU.mult)
                sqsf = fin.tile([128, 2, B], bf16)
                nc.vector.tensor_tensor(sqsf, hsfT_bf, hsfT_bf, op=ALU.mult)

                ssqff_ps = fpsum.tile([GPC, 1], f32)
                for c in range(2):
                    nc.tensor.matmul(
                        ssqff_ps, sqff[:, c, :], ones_col,
                        start=(c == 0), stop=(c == 1),
                    )
                ssqsf_ps = fpsum.tile([128, 4], f32)
                for jc in range(4):
                    for c in range(2):
                        nc.tensor.matmul(
                            ssqsf_ps[:, jc : jc + 1],
                            sqsf[:, c, jc * 128 : (jc + 1) * 128],
                            ones_col,
                            start=(c == 0), stop=(c == 1),
                        )

                ssqff_sb = fin.tile([GPC, 1], f32)
                nc.vector.tensor_copy(ssqff_sb, ssqff_ps)
                ssqsf_sb = fin.tile([128, 4], f32)
                nc.vector.tensor_copy(ssqsf_sb, ssqsf_ps)

                lnff = fin.tile([GPC, 1], f32)
                nc.scalar.activation(lnff, ssqff_sb, AF.Ln)
                invf2_f = fin.tile([GPC, 1], f32)
                nc.scalar.activation(
                    invf2_f, lnff, AF.Exp, scale=-0.5, bias=lntau_c[:GPC]
                )
                lnsf = fin.tile([128, 4], f32)
                nc.scalar.activation(lnsf, ssqsf_sb, AF.Ln)
                invs_f = fin.tile([128, 4], f32)
                nc.scalar.activation(invs_f, lnsf, AF.Exp, scale=-0.5)

                # replicated inv-norm rows for hs_final: colrep -> PE transpose
                crep_f = fin.tile([128, 4, 128], bf16)
                for jc in range(4):
                    nc.vector.tensor_scalar_mul(
                        crep_f[:, jc, :], ones_sq, invs_f[:, jc : jc + 1]
                    )
                repl_f = fpsum.tile([128, 4, 128], bf16)
                for jc in range(4):
                    nc.tensor.transpose(repl_f[:, jc, :], crep_f[:, jc, :], ident)

                hsfN = fin.tile([128, 2, B], bf16)
                for jc in range(4):
                    nc.vector.tensor_tensor(
                        hsfN[:, :, jc * 128 : (jc + 1) * 128],
                        hsfT_bf[:, :, jc * 128 : (jc + 1) * 128],
                        repl_f[:, jc, :].unsqueeze(1).to_broadcast((128, 2, 128)),
                        op=ALU.mult,
                    )

                sfin_ps = fpsum.tile([GPC, B], f32)
                for c in range(2):
                    nc.tensor.matmul(
                        sfin_ps, hffT_bf[:, c, :], hsfN[:, c, :],
                        start=(c == 0), stop=(c == 1),
                    )

                expf = fin.tile([GPC, B], bf16)
                rowsum_f = fin.tile([GPC, 1], f32)
                nc.scalar.activation(
                    expf, sfin_ps, AF.Exp, scale=invf2_f, accum_out=rowsum_f
                )

                # diag of own-graph block (cols 0:GPC after the host roll)
                dexp_f = fin.tile([GPC, 1], f32)
                maskd = fin.tile([GPC, GPC], bf16)
                nc.vector.tensor_tensor(
                    maskd, expf[:, :GPC], ident[:GPC, :GPC], op=ALU.mult
                )
                dscr = fin.tile([GPC, GPC], bf16)
                nc.vector.tensor_scalar(
                    dscr, maskd, 1.0, 0.0,
                    op0=ALU.mult, op1=ALU.add, accum_out=dexp_f,
                )

                recip_f = fin.tile([GPC, 1], f32)
                nc.vector.reciprocal(recip_f, dexp_f)
                ratio_f = fin.tile([GPC, 1], f32)
                nc.vector.tensor_tensor(ratio_f, rowsum_f, recip_f, op=ALU.mult)
                lg = fin.tile([GPC, 1], f32)
                nc.scalar.activation(lg, ratio_f, AF.Ln)
                nc.sync.dma_start(out=out_graph[:, :], in_=lg)

            # ---------------- node-level loss ----------------
            hfT_r = hfT[:, :].rearrange("(c p) n -> p c n", p=128)
            hsT_r = hsT[:, :].rearrange("(c p) n -> p c n", p=128)
            with (
                tc.tile_pool(
                    name="spsum", bufs=int(os.environ.get("K_SPSUM", "2")), space="PSUM"
                ) as spsum,
                tc.tile_pool(name="cpsum", bufs=2, space="PSUM") as cpsum,
                tc.tile_pool(name="rpsum", bufs=2, space="PSUM") as rpsum,
            ):
                for k in range(NCH):
                    ks = slice(k * CHG * NPER, (k + 1) * CHG * NPER)
                    cs = slice(k * CHG, (k + 1) * CHG)
                    hfT_k = loads.tile([128, 2, CHG, 128], bf16, tag="hfT")
                    nc.gpsimd.dma_start(out=hfT_k, in_=hfT_r[:, :, ks])
                    hsT_k = loads.tile([128, 2, CHG, 128], bf16, tag="hsT")
                    nc.gpsimd.dma_start(out=hsT_k, in_=hsT_r[:, :, ks])

                    # squares
                    sqf = work.tile([128, 2, CHG, 128], bf16, tag="sqf")
                    if SQF_ON_ACT:
                        nc.scalar.activation(sqf, hfT_k, AF.Square)
                    else:
                        nc.vector.tensor_tensor(sqf, hfT_k, hfT_k, op=ALU.mult)
                    sqs = work.tile([128, 2, CHG, 128], bf16, tag="sqs")
                    if SQS_ON_ACT:
                        nc.scalar.activation(sqs, hsT_k, AF.Square)
                    else:
                        nc.vector.tensor_tensor(sqs, hsT_k, hsT_k, op=ALU.mult)

                    # per-node ssq columns on TensorE (stationary=squared tile)
                    ssq_ps = cpsum.tile([128, 2, CHG], f32, tag="ssq")
                    for g in range(CHG):
                        for c in range(2):
                            nc.tensor.matmul(
                                ssq_ps[:, 0, g : g + 1], sqf[:, c, g, :], ones_col,
                                start=(c == 0), stop=(c == 1),
                            )
                        for c in range(2):
                            nc.tensor.matmul(
                                ssq_ps[:, 1, g : g + 1], sqs[:, c, g, :], ones_col,
                                start=(c == 0), stop=(c == 1),
                            )
                    nc.vector.tensor_copy(ssqf_s[:, cs], ssq_ps[:, 0, :])
                    nc.vector.tensor_copy(ssqs_s[:, cs], ssq_ps[:, 1, :])

                    # inv norms as columns (invf2 is the ACT exp scale AP)
                    nc.scalar.activation(ln_scr[:, cs], ssqf_s[:, cs], AF.Ln)
                    nc.scalar.activation(
                        invf2_col[:, cs], ln_scr[:, cs], AF.Exp,
                        scale=-0.5, bias=lntau_c,
                    )
                    nc.scalar.activation(ln_scr[:, cs], ssqs_s[:, cs], AF.Ln)
                    nc.scalar.activation(
                        invs_col[:, cs], ln_scr[:, cs], AF.Exp, scale=-0.5
                    )

                    # per q-group: replicated invs rows (colrep TSP + PE
                    # transpose -> bf16 psum), hs scale TT, Gram, exp,
                    # rowsum, diag
                    for q in range(CHG // QG):
                        qs = slice(q * QG, (q + 1) * QG)
                        crep = scr.tile([128, QG, 128], bf16, tag="crep")
                        for j in range(QG):
                            gg = k * CHG + q * QG + j
                            nc.vector.tensor_scalar_mul(
                                crep[:, j, :], ones_sq, invs_col[:, gg : gg + 1]
                            )
                        repl = rpsum.tile([128, QG, 128], bf16, tag="repl")
                        for j in range(QG):
                            nc.tensor.transpose(
                                repl[:, j, :], crep[:, j, :], ident
                            )

                        hsN = work.tile([128, 2, QG, 128], bf16, tag="hsN")
                        nc.vector.tensor_tensor(
                            hsN, hsT_k[:, :, qs, :],
                            repl.unsqueeze(1).to_broadcast((128, 2, QG, 128)),
                            op=ALU.mult,
                        )

                        s_ps = spsum.tile([128, QG, 128], f32, tag="s_ps")
                        for j in range(QG):
                            g = q * QG + j
                            for c in range(2):
                                nc.tensor.matmul(
                                    s_ps[:, j, :],
                                    hfT_k[:, c, g, :], hsN[:, c, j, :],
                                    start=(c == 0), stop=(c == 1),
                                )

                        exp_q = scr.tile([128, QG, 128], bf16, tag="expq")
                        for j in range(QG):
                            gg = k * CHG + q * QG + j
                            nc.scalar.activation(
                                exp_q[:, j, :], s_ps[:, j, :], AF.Exp,
                                scale=invf2_col[:, gg : gg + 1],
                            )
                        for j in range(QG):
                            gg = k * CHG + q * QG + j
                            ts_scr = scr.tile([128, 128], bf16, tag="tss")
                            nc.vector.tensor_scalar(
                                ts_scr, exp_q[:, j, :], 1.0, 0.0,
                                op0=ALU.mult, op1=ALU.add,
                                accum_out=rowsum_c[:, gg : gg + 1],
                            )
                        maskd = scr.tile([128, QG, 128], bf16, tag="maskd")
                        if DIAG_POOL:
                            nc.gpsimd.tensor_tensor(
                                maskd, exp_q, identx, op=ALU.mult
                            )
                        else:
                            nc.vector.tensor_tensor(
                                maskd, exp_q, identx, op=ALU.mult
                            )
                        for j in range(QG):
                            gg = k * CHG + q * QG + j
                            dts_scr = scr.tile([128, 128], bf16, tag="dtss")
                            nc.vector.tensor_scalar(
                                dts_scr, maskd[:, j, :], 1.0, 0.0,
                                op0=ALU.mult, op1=ALU.add,
                                accum_out=dexp_c[:, gg : gg + 1],
                            )

                recip_c = colsp.tile([128, GPC], f32)
                nc.vector.reciprocal(recip_c, dexp_c)
                ratio_c = colsp.tile([128, GPC], f32)
                nc.vector.tensor_tensor(ratio_c, rowsum_c, recip_c, op=ALU.mult)
                l_cols = colsp.tile([128, GPC], f32)
                nc.scalar.activation(l_cols, ratio_c, AF.Ln)
                nc.sync.dma_start(out=out_node[:, :], in_=l_cols)

    nc.compile()
    return nc


def _get_nc():
    if "nc" not in _CACHE:
        _CACHE["nc"] = _build()
    return _CACHE["nc"]


def _run(in_maps, **kwargs):
    from concourse.bass_utils import run_bass_kernel_spmd

    return run_bass_kernel_spmd(_get_nc(), in_maps, core_ids=list(range(NCORES)), **kwargs)


def make_in_maps(h_f_final, h_s_final, h_f, h_s):
    h_f = np.asarray(h_f, dtype=np.float32)
    h_s = np.asarray(h_s, dtype=np.float32)
    h_f_final = np.asarray(h_f_final, dtype=np.float32)
    h_s_final = np.asarray(h_s_final, dtype=np.float32)
    rows = GPC * NPER
    in_maps = []
    for c in range(NCORES):
        # Feature-major (transposed) per-core shards; h_s_final rolled so each
        # core's own graphs sit at columns 0:GPC (core-independent diag mask).
        in_maps.append(
            {
                "hfT": np.ascontiguousarray(h_f[c * rows : (c + 1) * rows].T),
                "hsT": np.ascontiguousarray(h_s[c * rows : (c + 1) * rows].T),
                "hffT": np.ascontiguousarray(h_f_final[c * GPC : (c + 1) * GPC].T),
                "hsfT": np.ascontiguousarray(
                    np.roll(h_s_final, -GPC * c, axis=0).T
                ),
            }
        )
    return in_maps


def finish(results):
    l_node = np.concatenate(
        [r["out_node"].astype(np.float64).mean(axis=0) for r in results]
    )
    l_graph = np.concatenate([r["out_graph"][:, 0].astype(np.float64) for r in results])
    lam1 = l_node.std() + 1e-6
    lam2 = l_graph.std() + 1e-6
    return np.float32(lam1 * l_node.mean() + lam2 * l_graph.mean())


def kernel(h_f_final, h_s_final, h_f, h_s, batch=None, **_unused):
    res = _run(make_in_maps(h_f_final, h_s_final, h_f, h_s))
    return finish(res.results)
